# revision 51
# baseline (speedup 1.0000x reference)
"""Trainium2 Bass kernel for nn_AdditiveCouplingLayer — v2: fp8 DoubleRow
matmuls + odd-only device I/O.

y = x; y[:, 1::2] += MLP(x[:, 0::2])  with a 512->1024->1024->512 relu MLP.

Data-parallel over 8 NeuronCores (batch 65536 -> 8192/core), weights
replicated. The even (conditioning) columns of y are exactly x's even
columns, so the device never sees them: the host sends the masked half
pre-transposed+quantized (mT) and the odd columns (xo), the device
returns only yo = xo + b3 + MLP(mT), and the host re-interleaves. This
roughly halves device HBM traffic (2.25MB vs 4.25MB per 512-row tile),
so the kernel is purely PE-bound at the fp8 DoubleRow roofline
(~512k PE cycles/core ~ 218us @2.4GHz).

Matmuls: fp8 e4m3 with DoubleRow perf mode (2 contraction rows/cycle,
2x fp16 throughput), fp32 PSUM accumulation; ACT quantizes the relu
activations to fp8 on the fly. Measured rel err ~4.7e-3 (gate 2e-2).
MODE "f16" keeps the fp16 1-row/cycle path (rel err ~3e-5) with full
x/y device I/O.
"""

import os
import sys

sys.path.insert(0, "/opt/trn_rl_repo")

import numpy as np

B, D, F, H = 65536, 1024, 512, 1024
NCORES = 8
BPC = B // NCORES  # rows per core
TB = 512  # batch tile (matmul free dim)
NBT = BPC // TB  # batch tiles per core
MODE = os.environ.get("BASS_COUPLING_MODE", "fp8")

_cache = {}


def _build(mode):
    import concourse.bacc as bacc
    import concourse.tile as tile
    import concourse.mybir as mybir

    dt = mybir.dt
    AF = mybir.ActivationFunctionType
    fp8 = mode == "fp8"
    adt = dt.float8e4 if fp8 else dt.float16  # activation/weight dtype
    DR = mybir.MatmulPerfMode.DoubleRow if fp8 else None

    nc = bacc.Bacc(
        "TRN2", target_bir_lowering=False, debug=False, num_devices=NCORES
    )

    # xo/yo travel as fp16: the residual values are O(1), fp16 rounding
    # adds ~1e-4 to the rel err (4.78e-3 total vs 2e-2 gate) and halves
    # the load+store ring traffic, shrinking the final store-drain tail.
    xo_d = nc.dram_tensor("xo", [BPC, F], dt.float16, kind="ExternalInput").ap()
    # mT is host-pre-tiled to [NBT*128, 4*TB]: row (bt*128 + p) holds the
    # batch-tile-bt slice for all 4 feature k-chunks, so each tile's load
    # is one DMA with 2KB contiguous runs per partition (vs 512B runs
    # when slicing a [F, BPC] layout).
    mT_d = nc.dram_tensor("mT", [NBT * 128, 4 * TB], adt, kind="ExternalInput").ap()
    w_d = {}
    for name, shape in (("w1", [F, H]), ("w2", [H, H]), ("w3", [H, F])):
        w_d[name] = nc.dram_tensor(name, shape, adt, kind="ExternalInput").ap()
    b1_d = nc.dram_tensor("b1m", [128, H // 128], dt.float32, kind="ExternalInput").ap()
    b2_d = nc.dram_tensor("b2m", [128, H // 128], dt.float32, kind="ExternalInput").ap()
    yo_d = nc.dram_tensor("yo", [BPC, F], dt.float16, kind="ExternalOutput").ap()

    with tile.TileContext(nc) as tc:
        with (
            tc.tile_pool(name="wpool", bufs=1) as wpool,
            tc.tile_pool(name="xpool", bufs=3) as xpool,
            tc.tile_pool(name="mpool", bufs=3) as mpool,
            tc.tile_pool(name="hpool", bufs=3) as hpool,
            tc.tile_pool(name="pmm", bufs=6, space="PSUM") as pmm,
            tc.tile_pool(name="pfin", bufs=2, space="PSUM") as pfin,
        ):
            # --- resident weights/biases ---
            def load_w(name, rows, cols, eng):
                """3-dim tile [128, nk, cols]; k-chunk k lives at
                [:, k, :]. Loaded as one DMA per k-PAIR so the first L2
                matmul waits only on its own pair, not the whole matrix
                (the whole-matrix completion lands ~4us later at startup
                when both rings compete for HBM)."""
                nk = rows // 128
                big = wpool.tile([128, nk, cols], adt, tag=name, name=name)
                src = w_d[name].rearrange("(k p) c -> p k c", p=128)
                for j in range(nk // 2):
                    eng.dma_start(
                        big[:, 2 * j : 2 * j + 2, :], src[:, 2 * j : 2 * j + 2, :]
                    )
                return big

            def load_b(name, ap, n):
                # host pre-transposes biases to [128, n/128] so this DMA is
                # contiguous (a "(m p) -> p m" rearrange here is a 4-byte-
                # element gather that takes ~10us and stalls the DMA ring)
                t = wpool.tile([128, n // 128], dt.float32, tag=name)
                nc.scalar.dma_start(t[:], ap[:])
                return t

            # PE warmup: junk matmuls on a zeroed scratch tile keep the PE
            # busy through its HAM activity window while the first real
            # DMAs are in flight, so real matmuls start at 2.4GHz.
            scratch = wpool.tile([128, TB], dt.float16, tag="scratch")
            nc.gpsimd.memset(scratch[:], 0.0)
            pwarm = pmm.tile([128, TB], dt.float32, tag="mm")
            for _ in range(12):
                nc.tensor.matmul(
                    pwarm[:], scratch[:, :128], scratch[:], start=True, stop=True
                )

            # Startup DMA order is the critical path: W1 + tile-0 mT go
            # on the sync queue; b1/W2/W3 ride the scalar queue from the
            # start (the scalar ring runs concurrently and L2's first
            # matmul at ~14us would otherwise stall on a late W2).
            # W1's per-k-chunk DMAs are issued by l1_tile(0) AFTER the
            # tile-0 mT load so the k-th matmul's operands arrive
            # progressively.
            w1t = wpool.tile([128, 4, H], adt, tag="w1")
            # W1 splits across BOTH rings so layer 1's operands arrive
            # concurrently: pair 1 is the scalar ring's first transfer
            # (W2 is needed ~6us later, biases later still), pair 0
            # follows tile-0's mT on the sync ring in l1_tile(0).
            nc.scalar.dma_start(
                w1t[:, 2:4, :],
                w_d["w1"].rearrange("(k p) c -> p k c", p=128)[:, 2:4, :],
            )
            w2t = load_w("w2", H, H, nc.scalar)
            w3t = load_w("w3", H, F, nc.scalar)
            b1t = load_b("b1t", b1_d, H)
            b2t = load_b("b2t", b2_d, H)

            def mm_group(psum, pairs, perf_mode=None):
                n = len(pairs)
                for i, (lhsT, rhs) in enumerate(pairs):
                    nc.tensor.matmul(
                        psum[:], lhsT, rhs, start=(i == 0), stop=(i == n - 1),
                        perf_mode=perf_mode,
                    )

            def layer(wt, hin, bt, nout, oname):
                """Transposed-space layer: out [128, nout/128, TB] fp8/fp16
                = relu(W.T@in + b). wt: [128, nk, cols] weight tile; hin:
                [128, nk, TB] activation tile."""
                nk = hin.shape[1]
                out = hpool.tile([128, nout // 128, TB], adt, tag=oname)
                for m in range(nout // 128):
                    p = pmm.tile([128, TB], dt.float32, tag="mm")
                    ms = slice(m * 128, (m + 1) * 128)
                    if fp8:
                        pairs = [
                            (wt[:, 2 * j : 2 * j + 2, ms], hin[:, 2 * j : 2 * j + 2, :])
                            for j in range(nk // 2)
                        ]
                    else:
                        pairs = [
                            (wt[:, k : k + 1, ms], hin[:, k : k + 1, :])
                            for k in range(nk)
                        ]
                    mm_group(p, pairs, perf_mode=DR)
                    nc.scalar.activation(
                        out[:, m : m + 1, :], p[:], AF.Relu, bias=bt[:, m : m + 1]
                    )
                return out

            def l1_tile(bt_i):
                """mT load + layer 1 for one batch tile (issued one tile
                ahead of layers 2/3 so the PE never stalls on the W2/W3
                arrival at startup, and mT is naturally prefetched)."""
                mT = mpool.tile([128, 4, TB], adt, tag="mbig", name="mbig")
                nc.sync.dma_start(
                    mT[:], mT_d[bt_i * 128 : (bt_i + 1) * 128, :]
                )
                if bt_i == 0:
                    nc.sync.dma_start(
                        w1t[:, 0:2, :],
                        w_d["w1"].rearrange("(k p) c -> p k c", p=128)[:, 0:2, :],
                    )
                return layer(w1t, mT, b1t, H, "h1")

            h1 = l1_tile(0)
            pending_stores = []
            for bt_i in range(NBT):
                r0 = bt_i * TB

                h1_next = l1_tile(bt_i + 1) if bt_i + 1 < NBT else None

                # yo stores ride the scalar HWDGE queue, deferred one
                # iteration (all 4 chunk adds are long done by then) and
                # combined into ONE 1MB DMA so the ACT sequencer pays the
                # ~0.6us issue cost once per tile instead of 4x.
                for rows, src in pending_stores:
                    nc.scalar.dma_start(rows, src)
                pending_stores = []

                # xo tile (odd columns + b3, fp16 — the residual). One
                # 3-dim-AP DMA brings all 4 row-chunks side by side.
                xobig = xpool.tile([128, 4, F], dt.float16, tag="xobig")
                nc.sync.dma_start(
                    xobig[:],
                    xo_d[r0 : r0 + TB, :].rearrange("(i p) c -> p i c", p=128),
                )
                h2 = layer(w2t, h1, b2t, H, "h2")

                # layer 3 in natural layout: stationary = h2 batch-slice,
                # moving = W3 tile  ->  psum[batch128, F]
                for i in range(4):
                    # final tile: a dedicated 2-buf PSUM pool forces
                    # chunk i+2's matmuls to wait on chunk i's drain, so
                    # the scheduler cannot push all four group-stops to
                    # the very end (which would serialize the adds+stores
                    # after the last matmul and stretch the kernel tail)
                    pool = pfin if bt_i == NBT - 1 else pmm
                    p = pool.tile([128, F], dt.float32, tag="mm")
                    bs = slice(i * 128, (i + 1) * 128)
                    if fp8:
                        pairs = [
                            (h2[:, 2 * j : 2 * j + 2, bs], w3t[:, 2 * j : 2 * j + 2, :])
                            for j in range(4)
                        ]
                    else:
                        pairs = [
                            (h2[:, k : k + 1, bs], w3t[:, k : k + 1, :])
                            for k in range(8)
                        ]
                    mm_group(p, pairs, perf_mode=DR)
                    rows = yo_d[r0 + i * 128 : r0 + (i + 1) * 128, :]
                    xoi = xobig[:, i : i + 1, :]
                    if bt_i == NBT - 1:
                        # final tile: one full-row add + one full-row
                        # store per chunk (2KB contiguous runs drain the
                        # ring ~4x faster than split 512B stores), with
                        # store queues alternated so chunk i+1's drain
                        # overlaps chunk i's. Only DVE/ACT can read PSUM,
                        # so all adds stay on the vector engine.
                        nc.vector.tensor_add(xoi, xoi, p[:])
                        eng = nc.sync if i % 2 == 0 else nc.scalar
                        eng.dma_start(rows[:], xoi)
                    else:
                        nc.vector.tensor_add(xoi, xoi, p[:])
                        pending_stores.append((rows[:], xoi))

                if h1_next is not None:
                    h1 = h1_next

    nc.compile()
    return nc


def _get(mode):
    if mode not in _cache:
        _cache[mode] = _build(mode)
    return _cache[mode]


def _in_maps(x, W1, b1, W2, b2, W3, b3):
    import ml_dtypes

    qdt = ml_dtypes.float8_e4m3 if MODE == "fp8" else np.float16

    ws = {
        name: np.asarray(w, np.float32).astype(qdt)
        for name, w in (("w1", W1), ("w2", W2), ("w3", W3))
    }

    common = dict(
        ws,
        b1m=np.ascontiguousarray(np.asarray(b1, np.float32).reshape(-1, 128).T),
        b2m=np.ascontiguousarray(np.asarray(b2, np.float32).reshape(-1, 128).T),
    )
    x = np.asarray(x, np.float32)
    b3f = np.asarray(b3, np.float32)
    in_maps = []
    for c in range(NCORES):
        xs = x[c * BPC : (c + 1) * BPC]
        masked_t = xs[:, 0::2].T.astype(qdt)  # [F, BPC] fp8
        # pre-tile to [NBT*128, 4*TB]: row (bt*128+p) = all 4 k-chunks of
        # batch-tile bt, so each device tile load is fully contiguous
        mt = np.ascontiguousarray(
            masked_t.reshape(4, 128, NBT, TB).transpose(2, 1, 0, 3)
        ).reshape(NBT * 128, 4 * TB)
        in_maps.append(
            dict(
                common,
                # b3 is folded into the residual here (one fused pass)
                # so the device never does the bias pre-add
                xo=(xs[:, 1::2] + b3f).astype(np.float16),
                mT=mt,
            )
        )
    return in_maps


def kernel(x, W1, b1, W2, b2, W3, b3):
    from concourse.bass_utils import run_bass_kernel_spmd

    nc = _get(MODE)
    x = np.asarray(x, np.float32)
    res = run_bass_kernel_spmd(
        nc, _in_maps(x, W1, b1, W2, b2, W3, b3), core_ids=list(range(NCORES))
    )
    y = np.empty((B, D), dtype=np.float32)
    y[:, 0::2] = x[:, 0::2]
    yo = np.concatenate([res.results[c]["yo"] for c in range(NCORES)], axis=0)
    y[:, 1::2] = yo.astype(np.float32)
    return y


# revision 52
# speedup vs baseline: 1.0236x; 1.0236x over previous
"""Trainium2 Bass kernel for nn_AdditiveCouplingLayer — v2: fp8 DoubleRow
matmuls + odd-only device I/O.

y = x; y[:, 1::2] += MLP(x[:, 0::2])  with a 512->1024->1024->512 relu MLP.

Data-parallel over 8 NeuronCores (batch 65536 -> 8192/core), weights
replicated. The even (conditioning) columns of y are exactly x's even
columns, so the device never sees them: the host sends the masked half
pre-transposed+quantized (mT) and the odd columns (xo), the device
returns only yo = xo + b3 + MLP(mT), and the host re-interleaves. This
roughly halves device HBM traffic (2.25MB vs 4.25MB per 512-row tile),
so the kernel is purely PE-bound at the fp8 DoubleRow roofline
(~512k PE cycles/core ~ 218us @2.4GHz).

Matmuls: fp8 e4m3 with DoubleRow perf mode (2 contraction rows/cycle,
2x fp16 throughput), fp32 PSUM accumulation; ACT quantizes the relu
activations to fp8 on the fly. Measured rel err ~4.7e-3 (gate 2e-2).
MODE "f16" keeps the fp16 1-row/cycle path (rel err ~3e-5) with full
x/y device I/O.
"""

import os
import sys

sys.path.insert(0, "/opt/trn_rl_repo")

import numpy as np

B, D, F, H = 65536, 1024, 512, 1024
NCORES = 8
BPC = B // NCORES  # rows per core
TB = 512  # batch tile (matmul free dim)
NBT = BPC // TB  # batch tiles per core
MODE = os.environ.get("BASS_COUPLING_MODE", "fp8")

_cache = {}


def _build(mode):
    import concourse.bacc as bacc
    import concourse.tile as tile
    import concourse.mybir as mybir

    dt = mybir.dt
    AF = mybir.ActivationFunctionType
    fp8 = mode == "fp8"
    adt = dt.float8e4 if fp8 else dt.float16  # activation/weight dtype
    DR = mybir.MatmulPerfMode.DoubleRow if fp8 else None

    nc = bacc.Bacc(
        "TRN2", target_bir_lowering=False, debug=False, num_devices=NCORES
    )

    # xo/yo travel as fp16: the residual values are O(1), fp16 rounding
    # adds ~1e-4 to the rel err (4.78e-3 total vs 2e-2 gate) and halves
    # the load+store ring traffic, shrinking the final store-drain tail.
    xo_d = nc.dram_tensor("xo", [BPC, F], dt.float16, kind="ExternalInput").ap()
    # mT is host-pre-tiled to [NBT*128, 4*TB]: row (bt*128 + p) holds the
    # batch-tile-bt slice for all 4 feature k-chunks, so each tile's load
    # is one DMA with 2KB contiguous runs per partition (vs 512B runs
    # when slicing a [F, BPC] layout).
    mT_d = nc.dram_tensor("mT", [NBT * 128, 4 * TB], adt, kind="ExternalInput").ap()
    w_d = {}
    for name, shape in (("w1", [F, H]), ("w2", [H, H]), ("w3", [H, F])):
        w_d[name] = nc.dram_tensor(name, shape, adt, kind="ExternalInput").ap()
    b1_d = nc.dram_tensor("b1m", [128, H // 128], dt.float32, kind="ExternalInput").ap()
    b2_d = nc.dram_tensor("b2m", [128, H // 128], dt.float32, kind="ExternalInput").ap()
    yo_d = nc.dram_tensor("yo", [BPC, F], dt.float16, kind="ExternalOutput").ap()

    with tile.TileContext(nc) as tc:
        with (
            tc.tile_pool(name="wpool", bufs=1) as wpool,
            tc.tile_pool(name="xpool", bufs=3) as xpool,
            tc.tile_pool(name="mpool", bufs=3) as mpool,
            tc.tile_pool(name="hpool", bufs=3) as hpool,
            tc.tile_pool(name="pmm", bufs=6, space="PSUM") as pmm,
            tc.tile_pool(name="pfin", bufs=2, space="PSUM") as pfin,
        ):
            # --- resident weights/biases ---
            def load_w(name, rows, cols, eng):
                """3-dim tile [128, nk, cols]; k-chunk k lives at
                [:, k, :]. Loaded as one DMA per k-PAIR so the first L2
                matmul waits only on its own pair, not the whole matrix
                (the whole-matrix completion lands ~4us later at startup
                when both rings compete for HBM)."""
                nk = rows // 128
                big = wpool.tile([128, nk, cols], adt, tag=name, name=name)
                src = w_d[name].rearrange("(k p) c -> p k c", p=128)
                for j in range(nk // 2):
                    eng.dma_start(
                        big[:, 2 * j : 2 * j + 2, :], src[:, 2 * j : 2 * j + 2, :]
                    )
                return big

            def load_b(name, ap, n):
                # host pre-transposes biases to [128, n/128] so this DMA is
                # contiguous (a "(m p) -> p m" rearrange here is a 4-byte-
                # element gather that takes ~10us and stalls the DMA ring)
                t = wpool.tile([128, n // 128], dt.float32, tag=name)
                nc.scalar.dma_start(t[:], ap[:])
                return t

            # PE warmup: junk matmuls on a zeroed scratch tile keep the PE
            # busy through its HAM activity window while the first real
            # DMAs are in flight, so real matmuls start at 2.4GHz.
            scratch = wpool.tile([128, TB], dt.float16, tag="scratch")
            nc.gpsimd.memset(scratch[:], 0.0)
            pwarm = pmm.tile([128, TB], dt.float32, tag="mm")
            for _ in range(12):
                nc.tensor.matmul(
                    pwarm[:], scratch[:, :128], scratch[:], start=True, stop=True
                )

            # Startup DMA order is the critical path: W1 + tile-0 mT go
            # on the sync queue; b1/W2/W3 ride the scalar queue from the
            # start (the scalar ring runs concurrently and L2's first
            # matmul at ~14us would otherwise stall on a late W2).
            # W1's per-k-chunk DMAs are issued by l1_tile(0) AFTER the
            # tile-0 mT load so the k-th matmul's operands arrive
            # progressively.
            w1t = wpool.tile([128, 4, H], adt, tag="w1")
            b1t = load_b("b1t", b1_d, H)
            b2t = load_b("b2t", b2_d, H)
            w2t = load_w("w2", H, H, nc.scalar)
            w3t = load_w("w3", H, F, nc.scalar)

            def mm_group(psum, pairs, perf_mode=None):
                n = len(pairs)
                for i, (lhsT, rhs) in enumerate(pairs):
                    nc.tensor.matmul(
                        psum[:], lhsT, rhs, start=(i == 0), stop=(i == n - 1),
                        perf_mode=perf_mode,
                    )

            def layer(wt, hin, bt, nout, oname):
                """Transposed-space layer: out [128, nout/128, TB] fp8/fp16
                = relu(W.T@in + b). wt: [128, nk, cols] weight tile; hin:
                [128, nk, TB] activation tile."""
                nk = hin.shape[1]
                out = hpool.tile([128, nout // 128, TB], adt, tag=oname)
                for m in range(nout // 128):
                    p = pmm.tile([128, TB], dt.float32, tag="mm")
                    ms = slice(m * 128, (m + 1) * 128)
                    if fp8:
                        pairs = [
                            (wt[:, 2 * j : 2 * j + 2, ms], hin[:, 2 * j : 2 * j + 2, :])
                            for j in range(nk // 2)
                        ]
                    else:
                        pairs = [
                            (wt[:, k : k + 1, ms], hin[:, k : k + 1, :])
                            for k in range(nk)
                        ]
                    mm_group(p, pairs, perf_mode=DR)
                    nc.scalar.activation(
                        out[:, m : m + 1, :], p[:], AF.Relu, bias=bt[:, m : m + 1]
                    )
                return out

            def l1_tile(bt_i):
                """mT load + layer 1 for one batch tile (issued one tile
                ahead of layers 2/3 so the PE never stalls on the W2/W3
                arrival at startup, and mT is naturally prefetched)."""
                mT = mpool.tile([128, 4, TB], adt, tag="mbig", name="mbig")
                nc.sync.dma_start(
                    mT[:], mT_d[bt_i * 128 : (bt_i + 1) * 128, :]
                )
                if bt_i == 0:
                    for k in range(4):
                        nc.sync.dma_start(
                            w1t[:, k : k + 1, :],
                            w_d["w1"][k * 128 : (k + 1) * 128, :],
                        )
                return layer(w1t, mT, b1t, H, "h1")

            h1 = l1_tile(0)
            pending_stores = []
            for bt_i in range(NBT):
                r0 = bt_i * TB

                h1_next = l1_tile(bt_i + 1) if bt_i + 1 < NBT else None

                # yo stores ride the scalar HWDGE queue, deferred one
                # iteration (all 4 chunk adds are long done by then) and
                # combined into ONE 1MB DMA so the ACT sequencer pays the
                # ~0.6us issue cost once per tile instead of 4x.
                for rows, src in pending_stores:
                    nc.scalar.dma_start(rows, src)
                pending_stores = []

                # xo tile (odd columns + b3, fp16 — the residual). One
                # 3-dim-AP DMA brings all 4 row-chunks side by side.
                xobig = xpool.tile([128, 4, F], dt.float16, tag="xobig")
                nc.sync.dma_start(
                    xobig[:],
                    xo_d[r0 : r0 + TB, :].rearrange("(i p) c -> p i c", p=128),
                )
                h2 = layer(w2t, h1, b2t, H, "h2")

                # layer 3 in natural layout: stationary = h2 batch-slice,
                # moving = W3 tile  ->  psum[batch128, F]
                for i in range(4):
                    # final tile: a dedicated 2-buf PSUM pool forces
                    # chunk i+2's matmuls to wait on chunk i's drain, so
                    # the scheduler cannot push all four group-stops to
                    # the very end (which would serialize the adds+stores
                    # after the last matmul and stretch the kernel tail)
                    pool = pfin if bt_i == NBT - 1 else pmm
                    p = pool.tile([128, F], dt.float32, tag="mm")
                    bs = slice(i * 128, (i + 1) * 128)
                    if fp8:
                        pairs = [
                            (h2[:, 2 * j : 2 * j + 2, bs], w3t[:, 2 * j : 2 * j + 2, :])
                            for j in range(4)
                        ]
                    else:
                        pairs = [
                            (h2[:, k : k + 1, bs], w3t[:, k : k + 1, :])
                            for k in range(8)
                        ]
                    mm_group(p, pairs, perf_mode=DR)
                    rows = yo_d[r0 + i * 128 : r0 + (i + 1) * 128, :]
                    xoi = xobig[:, i : i + 1, :]
                    if bt_i == NBT - 1:
                        # final tile: one full-row add + one full-row
                        # store per chunk (2KB contiguous runs drain the
                        # ring ~4x faster than split 512B stores), with
                        # store queues alternated so chunk i+1's drain
                        # overlaps chunk i's. Only DVE/ACT can read PSUM,
                        # so all adds stay on the vector engine.
                        nc.vector.tensor_add(xoi, xoi, p[:])
                        eng = nc.sync if i % 2 == 0 else nc.scalar
                        eng.dma_start(rows[:], xoi)
                    else:
                        nc.vector.tensor_add(xoi, xoi, p[:])
                        pending_stores.append((rows[:], xoi))

                if h1_next is not None:
                    h1 = h1_next

    nc.compile()
    return nc


def _get(mode):
    if mode not in _cache:
        _cache[mode] = _build(mode)
    return _cache[mode]


def _in_maps(x, W1, b1, W2, b2, W3, b3):
    import ml_dtypes

    qdt = ml_dtypes.float8_e4m3 if MODE == "fp8" else np.float16

    ws = {
        name: np.asarray(w, np.float32).astype(qdt)
        for name, w in (("w1", W1), ("w2", W2), ("w3", W3))
    }

    common = dict(
        ws,
        b1m=np.ascontiguousarray(np.asarray(b1, np.float32).reshape(-1, 128).T),
        b2m=np.ascontiguousarray(np.asarray(b2, np.float32).reshape(-1, 128).T),
    )
    x = np.asarray(x, np.float32)
    b3f = np.asarray(b3, np.float32)
    in_maps = []
    for c in range(NCORES):
        xs = x[c * BPC : (c + 1) * BPC]
        masked_t = xs[:, 0::2].T.astype(qdt)  # [F, BPC] fp8
        # pre-tile to [NBT*128, 4*TB]: row (bt*128+p) = all 4 k-chunks of
        # batch-tile bt, so each device tile load is fully contiguous
        mt = np.ascontiguousarray(
            masked_t.reshape(4, 128, NBT, TB).transpose(2, 1, 0, 3)
        ).reshape(NBT * 128, 4 * TB)
        in_maps.append(
            dict(
                common,
                # b3 is folded into the residual here (one fused pass)
                # so the device never does the bias pre-add
                xo=(xs[:, 1::2] + b3f).astype(np.float16),
                mT=mt,
            )
        )
    return in_maps


def kernel(x, W1, b1, W2, b2, W3, b3):
    from concourse.bass_utils import run_bass_kernel_spmd

    nc = _get(MODE)
    x = np.asarray(x, np.float32)
    res = run_bass_kernel_spmd(
        nc, _in_maps(x, W1, b1, W2, b2, W3, b3), core_ids=list(range(NCORES))
    )
    y = np.empty((B, D), dtype=np.float32)
    y[:, 0::2] = x[:, 0::2]
    yo = np.concatenate([res.results[c]["yo"] for c in range(NCORES)], axis=0)
    y[:, 1::2] = yo.astype(np.float32)
    return y


# revision 53
# speedup vs baseline: 1.0284x; 1.0047x over previous
"""Trainium2 Bass kernel for nn_AdditiveCouplingLayer — v2: fp8 DoubleRow
matmuls + odd-only device I/O.

y = x; y[:, 1::2] += MLP(x[:, 0::2])  with a 512->1024->1024->512 relu MLP.

Data-parallel over 8 NeuronCores (batch 65536 -> 8192/core), weights
replicated. The even (conditioning) columns of y are exactly x's even
columns, so the device never sees them: the host sends the masked half
pre-transposed+quantized (mT) and the odd columns (xo), the device
returns only yo = xo + b3 + MLP(mT), and the host re-interleaves. This
roughly halves device HBM traffic (2.25MB vs 4.25MB per 512-row tile),
so the kernel is purely PE-bound at the fp8 DoubleRow roofline
(~512k PE cycles/core ~ 218us @2.4GHz).

Matmuls: fp8 e4m3 with DoubleRow perf mode (2 contraction rows/cycle,
2x fp16 throughput), fp32 PSUM accumulation; ACT quantizes the relu
activations to fp8 on the fly. Measured rel err ~4.7e-3 (gate 2e-2).
MODE "f16" keeps the fp16 1-row/cycle path (rel err ~3e-5) with full
x/y device I/O.
"""

import os
import sys

sys.path.insert(0, "/opt/trn_rl_repo")

import numpy as np

B, D, F, H = 65536, 1024, 512, 1024
NCORES = 8
BPC = B // NCORES  # rows per core
TB = 512  # batch tile (matmul free dim)
NBT = BPC // TB  # batch tiles per core
MODE = os.environ.get("BASS_COUPLING_MODE", "fp8")

_cache = {}


def _build(mode):
    import concourse.bacc as bacc
    import concourse.tile as tile
    import concourse.mybir as mybir

    dt = mybir.dt
    AF = mybir.ActivationFunctionType
    fp8 = mode == "fp8"
    adt = dt.float8e4 if fp8 else dt.float16  # activation/weight dtype
    DR = mybir.MatmulPerfMode.DoubleRow if fp8 else None

    nc = bacc.Bacc(
        "TRN2", target_bir_lowering=False, debug=False, num_devices=NCORES
    )

    # xo/yo travel as fp16: the residual values are O(1), fp16 rounding
    # adds ~1e-4 to the rel err (4.78e-3 total vs 2e-2 gate) and halves
    # the load+store ring traffic, shrinking the final store-drain tail.
    xo_d = nc.dram_tensor("xo", [BPC, F], dt.float16, kind="ExternalInput").ap()
    # mT is host-pre-tiled to [NBT*128, 4*TB]: row (bt*128 + p) holds the
    # batch-tile-bt slice for all 4 feature k-chunks, so each tile's load
    # is one DMA with 2KB contiguous runs per partition (vs 512B runs
    # when slicing a [F, BPC] layout).
    mT_d = nc.dram_tensor("mT", [NBT * 128, 4 * TB], adt, kind="ExternalInput").ap()
    w_d = {}
    for name, shape in (("w1", [F, H]), ("w2", [H, H]), ("w3", [H, F])):
        w_d[name] = nc.dram_tensor(name, shape, adt, kind="ExternalInput").ap()
    b1_d = nc.dram_tensor("b1m", [128, H // 128], dt.float32, kind="ExternalInput").ap()
    b2_d = nc.dram_tensor("b2m", [128, H // 128], dt.float32, kind="ExternalInput").ap()
    yo_d = nc.dram_tensor("yo", [BPC, F], dt.float16, kind="ExternalOutput").ap()

    with tile.TileContext(nc) as tc:
        with (
            tc.tile_pool(name="wpool", bufs=1) as wpool,
            tc.tile_pool(name="xpool", bufs=3) as xpool,
            tc.tile_pool(name="mpool", bufs=3) as mpool,
            tc.tile_pool(name="hpool", bufs=3) as hpool,
            tc.tile_pool(name="pmm", bufs=6, space="PSUM") as pmm,
            tc.tile_pool(name="pfin", bufs=2, space="PSUM") as pfin,
        ):
            # --- resident weights/biases ---
            def load_w(name, rows, cols, eng):
                """3-dim tile [128, nk, cols]; k-chunk k lives at
                [:, k, :]. Loaded as one DMA per k-PAIR so the first L2
                matmul waits only on its own pair, not the whole matrix
                (the whole-matrix completion lands ~4us later at startup
                when both rings compete for HBM)."""
                nk = rows // 128
                big = wpool.tile([128, nk, cols], adt, tag=name, name=name)
                src = w_d[name].rearrange("(k p) c -> p k c", p=128)
                for j in range(nk // 2):
                    eng.dma_start(
                        big[:, 2 * j : 2 * j + 2, :], src[:, 2 * j : 2 * j + 2, :]
                    )
                return big

            def load_b(name, ap, n):
                # host pre-transposes biases to [128, n/128] so this DMA is
                # contiguous (a "(m p) -> p m" rearrange here is a 4-byte-
                # element gather that takes ~10us and stalls the DMA ring)
                t = wpool.tile([128, n // 128], dt.float32, tag=name)
                nc.scalar.dma_start(t[:], ap[:])
                return t

            # PE warmup: junk matmuls on a zeroed scratch tile keep the PE
            # busy through its HAM activity window while the first real
            # DMAs are in flight, so real matmuls start at 2.4GHz.
            scratch = wpool.tile([128, TB], dt.float16, tag="scratch")
            nc.gpsimd.memset(scratch[:], 0.0)
            pwarm = pmm.tile([128, TB], dt.float32, tag="mm")
            for _ in range(10):
                nc.tensor.matmul(
                    pwarm[:], scratch[:, :128], scratch[:], start=True, stop=True
                )

            # Startup DMA order is the critical path: W1 + tile-0 mT go
            # on the sync queue; b1/W2/W3 ride the scalar queue from the
            # start (the scalar ring runs concurrently and L2's first
            # matmul at ~14us would otherwise stall on a late W2).
            # W1's per-k-chunk DMAs are issued by l1_tile(0) AFTER the
            # tile-0 mT load so the k-th matmul's operands arrive
            # progressively.
            w1t = wpool.tile([128, 4, H], adt, tag="w1")
            b1t = load_b("b1t", b1_d, H)
            b2t = load_b("b2t", b2_d, H)
            w2t = load_w("w2", H, H, nc.scalar)
            w3t = load_w("w3", H, F, nc.scalar)

            def mm_group(psum, pairs, perf_mode=None):
                n = len(pairs)
                for i, (lhsT, rhs) in enumerate(pairs):
                    nc.tensor.matmul(
                        psum[:], lhsT, rhs, start=(i == 0), stop=(i == n - 1),
                        perf_mode=perf_mode,
                    )

            def layer(wt, hin, bt, nout, oname):
                """Transposed-space layer: out [128, nout/128, TB] fp8/fp16
                = relu(W.T@in + b). wt: [128, nk, cols] weight tile; hin:
                [128, nk, TB] activation tile."""
                nk = hin.shape[1]
                out = hpool.tile([128, nout // 128, TB], adt, tag=oname)
                for m in range(nout // 128):
                    p = pmm.tile([128, TB], dt.float32, tag="mm")
                    ms = slice(m * 128, (m + 1) * 128)
                    if fp8:
                        pairs = [
                            (wt[:, 2 * j : 2 * j + 2, ms], hin[:, 2 * j : 2 * j + 2, :])
                            for j in range(nk // 2)
                        ]
                    else:
                        pairs = [
                            (wt[:, k : k + 1, ms], hin[:, k : k + 1, :])
                            for k in range(nk)
                        ]
                    mm_group(p, pairs, perf_mode=DR)
                    nc.scalar.activation(
                        out[:, m : m + 1, :], p[:], AF.Relu, bias=bt[:, m : m + 1]
                    )
                return out

            def l1_tile(bt_i):
                """mT load + layer 1 for one batch tile (issued one tile
                ahead of layers 2/3 so the PE never stalls on the W2/W3
                arrival at startup, and mT is naturally prefetched)."""
                mT = mpool.tile([128, 4, TB], adt, tag="mbig", name="mbig")
                nc.sync.dma_start(
                    mT[:], mT_d[bt_i * 128 : (bt_i + 1) * 128, :]
                )
                if bt_i == 0:
                    for k in range(4):
                        nc.sync.dma_start(
                            w1t[:, k : k + 1, :],
                            w_d["w1"][k * 128 : (k + 1) * 128, :],
                        )
                return layer(w1t, mT, b1t, H, "h1")

            h1 = l1_tile(0)
            pending_stores = []
            for bt_i in range(NBT):
                r0 = bt_i * TB

                h1_next = l1_tile(bt_i + 1) if bt_i + 1 < NBT else None

                # yo stores ride the scalar HWDGE queue, deferred one
                # iteration (all 4 chunk adds are long done by then) and
                # combined into ONE 1MB DMA so the ACT sequencer pays the
                # ~0.6us issue cost once per tile instead of 4x.
                for rows, src in pending_stores:
                    nc.scalar.dma_start(rows, src)
                pending_stores = []

                # xo tile (odd columns + b3, fp16 — the residual). One
                # 3-dim-AP DMA brings all 4 row-chunks side by side.
                xobig = xpool.tile([128, 4, F], dt.float16, tag="xobig")
                nc.sync.dma_start(
                    xobig[:],
                    xo_d[r0 : r0 + TB, :].rearrange("(i p) c -> p i c", p=128),
                )
                h2 = layer(w2t, h1, b2t, H, "h2")

                # layer 3 in natural layout: stationary = h2 batch-slice,
                # moving = W3 tile  ->  psum[batch128, F]
                for i in range(4):
                    # final tile: a dedicated 2-buf PSUM pool forces
                    # chunk i+2's matmuls to wait on chunk i's drain, so
                    # the scheduler cannot push all four group-stops to
                    # the very end (which would serialize the adds+stores
                    # after the last matmul and stretch the kernel tail)
                    pool = pfin if bt_i == NBT - 1 else pmm
                    p = pool.tile([128, F], dt.float32, tag="mm")
                    bs = slice(i * 128, (i + 1) * 128)
                    if fp8:
                        pairs = [
                            (h2[:, 2 * j : 2 * j + 2, bs], w3t[:, 2 * j : 2 * j + 2, :])
                            for j in range(4)
                        ]
                    else:
                        pairs = [
                            (h2[:, k : k + 1, bs], w3t[:, k : k + 1, :])
                            for k in range(8)
                        ]
                    mm_group(p, pairs, perf_mode=DR)
                    rows = yo_d[r0 + i * 128 : r0 + (i + 1) * 128, :]
                    xoi = xobig[:, i : i + 1, :]
                    if bt_i == NBT - 1:
                        # final tile: one full-row add + one full-row
                        # store per chunk (2KB contiguous runs drain the
                        # ring ~4x faster than split 512B stores), with
                        # store queues alternated so chunk i+1's drain
                        # overlaps chunk i's. Only DVE/ACT can read PSUM,
                        # so all adds stay on the vector engine.
                        nc.vector.tensor_add(xoi, xoi, p[:])
                        eng = nc.sync if i % 2 == 0 else nc.scalar
                        eng.dma_start(rows[:], xoi)
                    else:
                        nc.vector.tensor_add(xoi, xoi, p[:])
                        pending_stores.append((rows[:], xoi))

                if h1_next is not None:
                    h1 = h1_next

    nc.compile()
    return nc


def _get(mode):
    if mode not in _cache:
        _cache[mode] = _build(mode)
    return _cache[mode]


def _in_maps(x, W1, b1, W2, b2, W3, b3):
    import ml_dtypes

    qdt = ml_dtypes.float8_e4m3 if MODE == "fp8" else np.float16

    ws = {
        name: np.asarray(w, np.float32).astype(qdt)
        for name, w in (("w1", W1), ("w2", W2), ("w3", W3))
    }

    common = dict(
        ws,
        b1m=np.ascontiguousarray(np.asarray(b1, np.float32).reshape(-1, 128).T),
        b2m=np.ascontiguousarray(np.asarray(b2, np.float32).reshape(-1, 128).T),
    )
    x = np.asarray(x, np.float32)
    b3f = np.asarray(b3, np.float32)
    in_maps = []
    for c in range(NCORES):
        xs = x[c * BPC : (c + 1) * BPC]
        masked_t = xs[:, 0::2].T.astype(qdt)  # [F, BPC] fp8
        # pre-tile to [NBT*128, 4*TB]: row (bt*128+p) = all 4 k-chunks of
        # batch-tile bt, so each device tile load is fully contiguous
        mt = np.ascontiguousarray(
            masked_t.reshape(4, 128, NBT, TB).transpose(2, 1, 0, 3)
        ).reshape(NBT * 128, 4 * TB)
        in_maps.append(
            dict(
                common,
                # b3 is folded into the residual here (one fused pass)
                # so the device never does the bias pre-add
                xo=(xs[:, 1::2] + b3f).astype(np.float16),
                mT=mt,
            )
        )
    return in_maps


def kernel(x, W1, b1, W2, b2, W3, b3):
    from concourse.bass_utils import run_bass_kernel_spmd

    nc = _get(MODE)
    x = np.asarray(x, np.float32)
    res = run_bass_kernel_spmd(
        nc, _in_maps(x, W1, b1, W2, b2, W3, b3), core_ids=list(range(NCORES))
    )
    y = np.empty((B, D), dtype=np.float32)
    y[:, 0::2] = x[:, 0::2]
    yo = np.concatenate([res.results[c]["yo"] for c in range(NCORES)], axis=0)
    y[:, 1::2] = yo.astype(np.float32)
    return y
